# revision 2
# baseline (speedup 1.0000x reference)
"""Trainium2 Bass kernel for SSL top-k contrastive loss (nn_SSLLoss1).

Math reduction: the reference's t0/t0 == 1, so
  pair_loss(a,b) = -N*log(1 + t1 + t2) with
  t1 = sum(exp(Saa)) - sum(exp(Saa*mask_a)) + self_a
  t2 = sum(exp(Sab)) - sum(exp(Sab*mask_b))
All terms are global scalars: only scalar reductions over the similarity
matrices are needed, never the [N,N] matrices themselves.

Sharding: rows of each embedding matrix across 8 cores (750 rows/core).
Each core computes its [750, 6000] similarity slabs (Saa, Sbb, Sab, Sba),
exp via ACT with fused row-accumulation (E sums), two-level top-k via
DVE max8 (threshold + top-30 value sum), and masked cross sums via a
single fused scalar_tensor_tensor ((X'_self >= theta) * X'_cross, accum).
Partial sums return to the host, which combines them in float64.
"""

import numpy as np
import ml_dtypes

N = 6000
D = 64
N_CORES = 8
ROWS_PER_CORE = N // N_CORES          # 750
ROW_CHUNKS = [(r * 128, min(128, ROWS_PER_CORE - r * 128))
              for r in range((ROWS_PER_CORE + 127) // 128)]   # 5x128 + 110
FCHUNK = 512
F_OFFS = [(k * FCHUNK, min(FCHUNK, N - k * FCHUNK)) for k in range((N + FCHUNK - 1) // FCHUNK)]
NF = len(F_OFFS)                      # 12
K_TOP = 30
TEMP = 50.0
SSL_TEMP = 0.1

# accE columns: 4 slabs x 12 f-chunks of exp-row-accumulators
# accV columns: 0=C2, 1=C3, 2=A2(top30 sum of Xaa), 3=B2(top30 sum of Xbb)
ACCE_COLS = 4 * NF                    # 48
ACCV_COLS = 8

_CACHE = {}


def _build_nc():
    import concourse.bass as bass
    import concourse.bacc as bacc
    import concourse.tile as tile
    from concourse import mybir
    from contextlib import ExitStack

    f32 = mybir.dt.float32
    bf16 = mybir.dt.bfloat16
    Exp = mybir.ActivationFunctionType.Exp
    Alu = mybir.AluOpType
    Ax = mybir.AxisListType

    nc = bacc.Bacc("TRN2", target_bir_lowering=False, debug=False,
                   num_devices=N_CORES)

    # full transposed normalized embeddings (rhs of matmuls) + per-core row
    # slabs of the same (lhsT of matmuls)
    ins_full = {}
    ins_slab = {}
    for name in ("u1", "u2", "i1", "i2"):
        ins_full[name] = nc.dram_tensor(f"{name}T", [D, N], bf16, kind="ExternalInput")
        ins_slab[name] = nc.dram_tensor(f"{name}Ts", [D, ROWS_PER_CORE], bf16,
                                        kind="ExternalInput")
    accE_out = nc.dram_tensor("accE_out", [2, len(ROW_CHUNKS), 128, ACCE_COLS],
                              f32, kind="ExternalOutput")
    accV_out = nc.dram_tensor("accV_out", [2, len(ROW_CHUNKS), 128, ACCV_COLS],
                              f32, kind="ExternalOutput")

    groups = [("u1", "u2"), ("i1", "i2")]

    with tile.TileContext(nc) as tc, ExitStack() as ctx:
        inpool = ctx.enter_context(tc.tile_pool(name="inputs", bufs=1))
        psum = ctx.enter_context(tc.tile_pool(name="psum", bufs=6,
                                              space=bass.MemorySpace.PSUM))
        xpool = ctx.enter_context(tc.tile_pool(name="xbuf", bufs=2))
        spool = ctx.enter_context(tc.tile_pool(name="small", bufs=2))
        apool = ctx.enter_context(tc.tile_pool(name="accs", bufs=2))

        # load all inputs into SBUF once
        sb_full = {}
        sb_slab = {}
        for name in ("u1", "u2", "i1", "i2"):
            tf = inpool.tile([D, N], bf16, tag=f"full_{name}")
            nc.sync.dma_start(tf[:], ins_full[name][:])
            sb_full[name] = tf
            tsl = inpool.tile([D, ROWS_PER_CORE], bf16, tag=f"slab_{name}")
            nc.sync.dma_start(tsl[:], ins_slab[name][:])
            sb_slab[name] = tsl

        for gi, (a, b) in enumerate(groups):
            for ri, (r0, rows) in enumerate(ROW_CHUNKS):
                lhs_a = sb_slab[a][:, r0:r0 + rows]
                lhs_b = sb_slab[b][:, r0:r0 + rows]
                accE = apool.tile([128, ACCE_COLS], f32, tag="accE")
                accV = apool.tile([128, ACCV_COLS], f32, tag="accV")

                # slab order: Xaa, Xbb, Xab, Xba
                slabs = [(lhs_a, sb_full[a]), (lhs_b, sb_full[b]),
                         (lhs_a, sb_full[b]), (lhs_b, sb_full[a])]
                X = []
                for si, (lh, rh) in enumerate(slabs):
                    xt = xpool.tile([128, N], bf16, tag=f"X{si}")
                    for k, (f0, fw) in enumerate(F_OFFS):
                        ps = psum.tile([128, FCHUNK], f32, tag="ps")
                        nc.tensor.matmul(ps[:rows, :fw], lh, rh[:, f0:f0 + fw],
                                         start=True, stop=True)
                        nc.scalar.activation(
                            xt[:rows, f0:f0 + fw], ps[:rows, :fw], Exp,
                            accum_out=accE[:rows, si * NF + k: si * NF + k + 1])
                    X.append(xt)

                # two-level top-k on Xaa and Xbb -> theta_mid + top30 sum
                thetas = []
                for ti in range(2):
                    xt = X[ti]
                    cand = spool.tile([128, 8 * NF], bf16, tag=f"cand{ti}")
                    for k, (f0, fw) in enumerate(F_OFFS):
                        nc.vector.max(cand[:rows, k * 8:(k + 1) * 8],
                                      xt[:rows, f0:f0 + fw])
                    gbuf = spool.tile([128, 32], bf16, tag=f"gbuf{ti}")
                    t1b = spool.tile([128, 8 * NF], bf16, tag=f"mr{ti}_0")
                    t2b = spool.tile([128, 8 * NF], bf16, tag=f"mr{ti}_1")
                    t3b = spool.tile([128, 8 * NF], bf16, tag=f"mr{ti}_2")
                    NEG = -3.0e38
                    nc.vector.max(gbuf[:rows, 0:8], cand[:rows, :])
                    nc.vector.match_replace(t1b[:rows, :], gbuf[:rows, 0:8],
                                            cand[:rows, :], NEG)
                    nc.vector.max(gbuf[:rows, 8:16], t1b[:rows, :])
                    nc.vector.match_replace(t2b[:rows, :], gbuf[:rows, 8:16],
                                            t1b[:rows, :], NEG)
                    nc.vector.max(gbuf[:rows, 16:24], t2b[:rows, :])
                    nc.vector.match_replace(t3b[:rows, :], gbuf[:rows, 16:24],
                                            t2b[:rows, :], NEG)
                    nc.vector.max(gbuf[:rows, 24:32], t3b[:rows, :])
                    # top30 sum -> accV col 2+ti
                    nc.vector.reduce_sum(accV[:rows, 2 + ti:3 + ti],
                                         gbuf[:rows, 0:30], axis=Ax.X)
                    # theta_mid = (v30 + v31) / 2, f32
                    tsum = spool.tile([128, 1], f32, tag=f"tsum{ti}")
                    nc.vector.tensor_add(tsum[:rows, :], gbuf[:rows, 29:30],
                                         gbuf[:rows, 30:31])
                    tmid = spool.tile([128, 1], f32, tag=f"tmid{ti}")
                    nc.vector.tensor_scalar_mul(tmid[:rows, :], tsum[:rows, :], 0.5)
                    thetas.append(tmid)

                # fused masked cross sums:
                # C2 = sum((Xbb >= theta_b) * Xab), C3 = sum((Xaa >= theta_a) * Xba)
                dummy = xpool.tile([128, N], bf16, tag="dummy")
                nc.vector.scalar_tensor_tensor(
                    dummy[:rows, :], X[1][:rows, :], thetas[1][:rows, :],
                    X[2][:rows, :], Alu.is_ge, Alu.mult,
                    accum_out=accV[:rows, 0:1])
                dummy2 = xpool.tile([128, N], bf16, tag="dummy")
                nc.vector.scalar_tensor_tensor(
                    dummy2[:rows, :], X[0][:rows, :], thetas[0][:rows, :],
                    X[3][:rows, :], Alu.is_ge, Alu.mult,
                    accum_out=accV[:rows, 1:2])

                nc.sync.dma_start(accE_out[gi, ri], accE[:])
                nc.sync.dma_start(accV_out[gi, ri], accV[:])

    nc.compile()
    return nc


def _normalize64(x):
    x = np.asarray(x, np.float64)
    n = np.sqrt((x * x).sum(axis=1, keepdims=True))
    return x / np.maximum(n, 1e-12)


def kernel(uemb1, uemb2, iemb1, iemb2):
    from concourse.bass_utils import run_bass_kernel_spmd

    if "nc" not in _CACHE:
        _CACHE["nc"] = _build_nc()
    nc = _CACHE["nc"]

    bf = ml_dtypes.bfloat16
    norm = {k: _normalize64(v) for k, v in
            (("u1", uemb1), ("u2", uemb2), ("i1", iemb1), ("i2", iemb2))}
    selfs = {k: np.exp((v * v) / SSL_TEMP).sum(dtype=np.float64)
             for k, v in norm.items()}
    full_T = {k: np.ascontiguousarray(v.astype(np.float32).astype(bf).T)
              for k, v in norm.items()}

    in_maps = []
    for c in range(N_CORES):
        sl = slice(c * ROWS_PER_CORE, (c + 1) * ROWS_PER_CORE)
        m = {}
        for k in ("u1", "u2", "i1", "i2"):
            m[f"{k}T"] = full_T[k]
            m[f"{k}Ts"] = np.ascontiguousarray(full_T[k][:, sl])
        in_maps.append(m)

    res = run_bass_kernel_spmd(nc, in_maps, list(range(N_CORES))).results

    # host combine in f64
    E = np.zeros((2, 4))   # [group, slab] slab order: aa, bb, ab, ba
    C2 = np.zeros(2)
    C3 = np.zeros(2)
    A2 = np.zeros(2)
    B2 = np.zeros(2)
    for c in range(N_CORES):
        accE = np.asarray(res[c]["accE_out"], np.float64)   # [2,6,128,48]
        accV = np.asarray(res[c]["accV_out"], np.float64)   # [2,6,128,8]
        for ri, (r0, rows) in enumerate(ROW_CHUNKS):
            e = accE[:, ri, :rows, :]                        # [2, rows, 48]
            v = accV[:, ri, :rows, :]
            for si in range(4):
                E[:, si] += e[:, :, si * NF:(si + 1) * NF].sum(axis=(1, 2))
            C2 += v[:, :, 0].sum(axis=1)
            C3 += v[:, :, 1].sum(axis=1)
            A2 += v[:, :, 2].sum(axis=1)
            B2 += v[:, :, 3].sum(axis=1)

    corr = float(N) * N - float(K_TOP) * N    # exp(0)=1 entries outside mask
    losses = []
    for gi, (a, b) in enumerate((("u1", "u2"), ("i1", "i2"))):
        t1 = E[gi, 0] - (A2[gi] + corr) + selfs[a]
        t2 = E[gi, 2] - (C2[gi] + corr)
        losses.append(-N * np.log(1.0 + t1 + t2))
        t1b = E[gi, 1] - (B2[gi] + corr) + selfs[b]
        t2b = E[gi, 3] - (C3[gi] + corr)
        losses.append(-N * np.log(1.0 + t1b + t2b))

    total = (losses[0] + losses[1] + losses[2] + losses[3]) / 4.0
    return np.float32(total)


# revision 10
# speedup vs baseline: 1.1932x; 1.1932x over previous
"""Trainium2 Bass kernel for SSL top-k contrastive loss (nn_SSLLoss1).

Math reduction: the reference's t0/t0 == 1, so
  pair_loss(a,b) = -N*log(1 + t1 + t2) with
  t1 = sum(exp(Saa)) - sum(exp(Saa*mask_a)) + self_a
  t2 = sum(exp(Sab)) - sum(exp(Sab*mask_b))
All terms are global scalars: only scalar reductions over the similarity
matrices are needed, never the [N,N] matrices themselves.

Sharding: rows of each embedding matrix across 8 cores (750 rows/core).
Each core computes its [750, 6000] similarity slabs (Saa, Sbb, Sab, Sba),
exp via ACT with fused row-accumulation (E sums), two-level top-k via
DVE max8 (threshold + top-30 value sum), and masked cross sums via a
single fused scalar_tensor_tensor ((X'_self >= theta) * X'_cross, accum).
Partial sums return to the host, which combines them in float64.
"""

import os

import numpy as np
import ml_dtypes

STT_ENGINE = os.environ.get("K_STT_ENGINE", "gpsimd")   # "vector" | "gpsimd"

N = 6000
D = 64
N_CORES = 8
ROWS_PER_CORE = N // N_CORES          # 750
ROW_CHUNKS = [(r * 128, min(128, ROWS_PER_CORE - r * 128))
              for r in range((ROWS_PER_CORE + 127) // 128)]   # 5x128 + 110
FCHUNK = 512
F_OFFS = [(k * FCHUNK, min(FCHUNK, N - k * FCHUNK)) for k in range((N + FCHUNK - 1) // FCHUNK)]
NF = len(F_OFFS)                      # 12
# PSUM tiles span 4 banks (2048 f32); one ACT exp+accum per tile
PCHUNK = 2048
P_OFFS = [(k * PCHUNK, min(PCHUNK, N - k * PCHUNK)) for k in range((N + PCHUNK - 1) // PCHUNK)]
NP = len(P_OFFS)                      # 3
K_TOP = 30
TEMP = 50.0
SSL_TEMP = 0.1

# accE columns: 4 slabs x 3 psum-chunks of exp-row-accumulators
# accV columns: 0=C2, 1=C3, 2=A2(top30 sum of Xaa), 3=B2(top30 sum of Xbb)
ACCE_COLS = 4 * NP                    # 12
ACCV_COLS = 8

_CACHE = {}


def _build_nc():
    import concourse.bass as bass
    import concourse.bacc as bacc
    import concourse.tile as tile
    from concourse import mybir
    from contextlib import ExitStack

    f32 = mybir.dt.float32
    bf16 = mybir.dt.bfloat16
    Exp = mybir.ActivationFunctionType.Exp
    Alu = mybir.AluOpType
    Ax = mybir.AxisListType

    nc = bacc.Bacc("TRN2", target_bir_lowering=False, debug=False,
                   num_devices=N_CORES)

    # full transposed normalized embeddings (rhs of matmuls) + per-core row
    # slabs of the same (lhsT of matmuls)
    ins_full = {}
    ins_slab = {}
    for name in ("u1", "u2", "i1", "i2"):
        ins_full[name] = nc.dram_tensor(f"{name}T", [D, N], bf16, kind="ExternalInput")
        ins_slab[name] = nc.dram_tensor(f"{name}Ts", [D, ROWS_PER_CORE], bf16,
                                        kind="ExternalInput")
    accE_out = nc.dram_tensor("accE_out", [2, len(ROW_CHUNKS), 128, ACCE_COLS],
                              f32, kind="ExternalOutput")
    accV_out = nc.dram_tensor("accV_out", [2, len(ROW_CHUNKS), 128, ACCV_COLS],
                              f32, kind="ExternalOutput")

    groups = [("u1", "u2"), ("i1", "i2")]

    with tile.TileContext(nc) as tc, ExitStack() as ctx:
        inpool = ctx.enter_context(tc.tile_pool(name="inputs", bufs=1))
        psum = ctx.enter_context(tc.tile_pool(name="psum", bufs=2,
                                              space=bass.MemorySpace.PSUM))
        xpool = ctx.enter_context(tc.tile_pool(name="xbuf", bufs=2))
        spool = ctx.enter_context(tc.tile_pool(name="small", bufs=2))
        apool = ctx.enter_context(tc.tile_pool(name="accs", bufs=2))

        # load all inputs into SBUF once
        sb_full = {}
        sb_slab = {}
        for name in ("u1", "u2", "i1", "i2"):
            tf = inpool.tile([D, N], bf16, tag=f"full_{name}")
            nc.sync.dma_start(tf[:], ins_full[name][:])
            sb_full[name] = tf
            tsl = inpool.tile([D, ROWS_PER_CORE], bf16, tag=f"slab_{name}")
            nc.sync.dma_start(tsl[:], ins_slab[name][:])
            sb_slab[name] = tsl

        for gi, (a, b) in enumerate(groups):
            for ri, (r0, rows) in enumerate(ROW_CHUNKS):
                lhs_a = sb_slab[a][:, r0:r0 + rows]
                lhs_b = sb_slab[b][:, r0:r0 + rows]
                accE = apool.tile([128, ACCE_COLS], f32, tag="accE")
                accV = apool.tile([128, ACCV_COLS], f32, tag="accV")

                # slab order: Xaa, Xbb, Xab, Xba
                slabs = [(lhs_a, sb_full[a]), (lhs_b, sb_full[b]),
                         (lhs_a, sb_full[b]), (lhs_b, sb_full[a])]
                X = []
                for si, (lh, rh) in enumerate(slabs):
                    xt = xpool.tile([128, N], bf16, tag=f"X{si}")
                    for p, (p0, pw) in enumerate(P_OFFS):
                        ps = psum.tile([128, PCHUNK], f32, tag="ps")
                        for f0 in range(0, pw, FCHUNK):
                            fw = min(FCHUNK, pw - f0)
                            nc.tensor.matmul(ps[:rows, f0:f0 + fw], lh,
                                             rh[:, p0 + f0:p0 + f0 + fw],
                                             start=True, stop=True)
                        nc.scalar.activation(
                            xt[:rows, p0:p0 + pw], ps[:rows, :pw], Exp,
                            accum_out=accE[:rows, si * NP + p: si * NP + p + 1])
                    X.append(xt)

                # two-level top-k on Xaa and Xbb -> theta_mid + top30 sum
                thetas = []
                for ti in range(2):
                    xt = X[ti]
                    cand = spool.tile([128, 8 * NF], bf16, tag=f"cand{ti}")
                    for k, (f0, fw) in enumerate(F_OFFS):
                        nc.vector.max(cand[:rows, k * 8:(k + 1) * 8],
                                      xt[:rows, f0:f0 + fw])
                    gbuf = spool.tile([128, 32], bf16, tag=f"gbuf{ti}")
                    t1b = spool.tile([128, 8 * NF], bf16, tag=f"mr{ti}_0")
                    t2b = spool.tile([128, 8 * NF], bf16, tag=f"mr{ti}_1")
                    t3b = spool.tile([128, 8 * NF], bf16, tag=f"mr{ti}_2")
                    NEG = -3.0e38
                    nc.vector.max(gbuf[:rows, 0:8], cand[:rows, :])
                    nc.vector.match_replace(t1b[:rows, :], gbuf[:rows, 0:8],
                                            cand[:rows, :], NEG)
                    nc.vector.max(gbuf[:rows, 8:16], t1b[:rows, :])
                    nc.vector.match_replace(t2b[:rows, :], gbuf[:rows, 8:16],
                                            t1b[:rows, :], NEG)
                    nc.vector.max(gbuf[:rows, 16:24], t2b[:rows, :])
                    nc.vector.match_replace(t3b[:rows, :], gbuf[:rows, 16:24],
                                            t2b[:rows, :], NEG)
                    nc.vector.max(gbuf[:rows, 24:32], t3b[:rows, :])
                    # top30 sum -> accV col 2+ti
                    nc.vector.reduce_sum(accV[:rows, 2 + ti:3 + ti],
                                         gbuf[:rows, 0:30], axis=Ax.X)
                    # theta_mid = (v30 + v31) / 2, f32
                    tsum = spool.tile([128, 1], f32, tag=f"tsum{ti}")
                    nc.vector.tensor_add(tsum[:rows, :], gbuf[:rows, 29:30],
                                         gbuf[:rows, 30:31])
                    tmid = spool.tile([128, 1], f32, tag=f"tmid{ti}")
                    nc.vector.tensor_scalar_mul(tmid[:rows, :], tsum[:rows, :], 0.5)
                    thetas.append(tmid)

                # fused masked cross sums:
                # C2 = sum((Xbb >= theta_b) * Xab), C3 = sum((Xaa >= theta_a) * Xba)
                stt_eng = nc.gpsimd if STT_ENGINE == "gpsimd" else nc.vector
                dummy = xpool.tile([128, N], bf16, tag="dummy")
                stt_eng.scalar_tensor_tensor(
                    dummy[:rows, :], X[1][:rows, :], thetas[1][:rows, :],
                    X[2][:rows, :], Alu.is_ge, Alu.mult,
                    accum_out=accV[:rows, 0:1])
                dummy2 = xpool.tile([128, N], bf16, tag="dummy")
                stt_eng.scalar_tensor_tensor(
                    dummy2[:rows, :], X[0][:rows, :], thetas[0][:rows, :],
                    X[3][:rows, :], Alu.is_ge, Alu.mult,
                    accum_out=accV[:rows, 1:2])

                nc.sync.dma_start(accE_out[gi, ri], accE[:])
                nc.sync.dma_start(accV_out[gi, ri], accV[:])

    nc.compile()
    return nc


def _normalize64(x):
    x = np.asarray(x, np.float64)
    n = np.sqrt((x * x).sum(axis=1, keepdims=True))
    return x / np.maximum(n, 1e-12)


def kernel(uemb1, uemb2, iemb1, iemb2):
    from concourse.bass_utils import run_bass_kernel_spmd

    if "nc" not in _CACHE:
        _CACHE["nc"] = _build_nc()
    nc = _CACHE["nc"]

    bf = ml_dtypes.bfloat16
    norm = {k: _normalize64(v) for k, v in
            (("u1", uemb1), ("u2", uemb2), ("i1", iemb1), ("i2", iemb2))}
    selfs = {k: np.exp((v * v) / SSL_TEMP).sum(dtype=np.float64)
             for k, v in norm.items()}
    full_T = {k: np.ascontiguousarray(v.astype(np.float32).astype(bf).T)
              for k, v in norm.items()}

    in_maps = []
    for c in range(N_CORES):
        sl = slice(c * ROWS_PER_CORE, (c + 1) * ROWS_PER_CORE)
        m = {}
        for k in ("u1", "u2", "i1", "i2"):
            m[f"{k}T"] = full_T[k]
            m[f"{k}Ts"] = np.ascontiguousarray(full_T[k][:, sl])
        in_maps.append(m)

    res = run_bass_kernel_spmd(nc, in_maps, list(range(N_CORES))).results

    # host combine in f64
    E = np.zeros((2, 4))   # [group, slab] slab order: aa, bb, ab, ba
    C2 = np.zeros(2)
    C3 = np.zeros(2)
    A2 = np.zeros(2)
    B2 = np.zeros(2)
    for c in range(N_CORES):
        accE = np.asarray(res[c]["accE_out"], np.float64)   # [2,6,128,48]
        accV = np.asarray(res[c]["accV_out"], np.float64)   # [2,6,128,8]
        for ri, (r0, rows) in enumerate(ROW_CHUNKS):
            e = accE[:, ri, :rows, :]                        # [2, rows, 48]
            v = accV[:, ri, :rows, :]
            for si in range(4):
                E[:, si] += e[:, :, si * NP:(si + 1) * NP].sum(axis=(1, 2))
            C2 += v[:, :, 0].sum(axis=1)
            C3 += v[:, :, 1].sum(axis=1)
            A2 += v[:, :, 2].sum(axis=1)
            B2 += v[:, :, 3].sum(axis=1)

    corr = float(N) * N - float(K_TOP) * N    # exp(0)=1 entries outside mask
    losses = []
    for gi, (a, b) in enumerate((("u1", "u2"), ("i1", "i2"))):
        t1 = E[gi, 0] - (A2[gi] + corr) + selfs[a]
        t2 = E[gi, 2] - (C2[gi] + corr)
        losses.append(-N * np.log(1.0 + t1 + t2))
        t1b = E[gi, 1] - (B2[gi] + corr) + selfs[b]
        t2b = E[gi, 3] - (C3[gi] + corr)
        losses.append(-N * np.log(1.0 + t1b + t2b))

    total = (losses[0] + losses[1] + losses[2] + losses[3]) / 4.0
    return np.float32(total)
